# revision 1
# baseline (speedup 1.0000x reference)
"""BERT layer (B=8, S=512, H=768, NH=12, DH=64, FF=3072) on 8 Trainium2 cores.

Strategy: data-parallel over batch (1 batch element per core, no collectives).
On-chip layout is feature-major ("transposed"): activations live as
X^T [H partitions, S free] so every matmul consumes the previous one's output
without any transposes:

  Q^T = Wq^T @ X^T          (lhsT = Wq block, rhs = X^T chunk)
  K^T = Wk^T @ X^T
  V    = X @ Wv             (token-major: lhsT = X^T block, rhs = Wv chunk)
  S^T  = K @ Q^T            (keys on partitions -> softmax sum via ones-matmul)
  P^T  = exp(S^T/8 + maskbias) * recip(sum)   (denominator matmul uses an
         all-ones [128,64] stationary so it directly yields broadcast rows)
  ctx^T = V^T @ P^T         (lhsT = V token-major slice)
  attn^T = Wo^T @ ctx^T ; +bias +residual ; LN1 (stats via ones-matmuls)
  ff1^T = W1^T @ h1^T ; gelu ; ff2^T = W2^T @ gelu^T ; +residual ; LN2

Matmul dtypes: f32r (full-rate fp32) for the QKV/S^T/Wo path, bf16 for the
attention-prob/V contraction and both FFN matmuls (fp32 PSUM accumulate).
All softmax/LN/residual arithmetic is fp32.

Bias folding: bv is folded into bo on the host (softmax rows sum to 1, so
(P @ (V + 1 bv^T)) @ Wo + bo == (P @ V) @ Wo + (bv @ Wo + bo)).
The 1/sqrt(DH) scale and the additive attention mask are folded into the
exp() activation's scale/bias operands.
"""

from contextlib import ExitStack

import numpy as np
import ml_dtypes

import concourse.bass as bass
from concourse import bacc
import concourse.tile as tile
from concourse import mybir
from concourse.bass_utils import run_bass_kernel_spmd

F32 = mybir.dt.float32
F32R = mybir.dt.float32r
BF16 = mybir.dt.bfloat16
AF = mybir.ActivationFunctionType
ALU = mybir.AluOpType

B, S, H, NH, DH, FF = 8, 512, 768, 12, 64, 3072
EPS = 1e-3
CH = H // 128   # 6 hidden chunks
CF = FF // 128  # 24 ff chunks
T = S // 128    # 4 token/key chunks
NP = NH // 2    # 6 head pairs

# consts tile column map: [128, NCONST]
BQ, BK, BO, L1G, L1B, B2, L2G, L2B = 0, 6, 12, 18, 24, 30, 36, 42
B1 = 48          # 24 cols
MB = B1 + CF     # 4 cols of mask bias
NCONST = MB + T


def ts(i, n):
    return slice(i * n, (i + 1) * n)


def build_nc(gelu_mode="hw", repeats=1):
    """gelu_mode: "hw" uses the ACT engine's native exact Gelu; "tanh" emits an
    explicit tanh-approximation (CoreSim does not implement Gelu)."""
    nc = bacc.Bacc("TRN2", target_bir_lowering=False, debug=False)

    xT_d = nc.declare_dram_parameter("xT", [H, S], F32R, isOutput=False)
    wqb_d = nc.declare_dram_parameter("wqb", [CH, 128, CH, 128], F32R,
                                      isOutput=False)
    wkb_d = nc.declare_dram_parameter("wkb", [CH, 128, CH, 128], F32R,
                                      isOutput=False)
    wv_d = nc.declare_dram_parameter("wv", [H, H], F32R, isOutput=False)
    wo_d = nc.declare_dram_parameter("wo", [H, H], F32R, isOutput=False)
    w1_d = nc.declare_dram_parameter("w1b", [CF, 128, CH, 128], BF16, isOutput=False)
    w2_d = nc.declare_dram_parameter("w2b", [CH, 128, CF, 128], BF16, isOutput=False)
    c_d = nc.declare_dram_parameter("consts", [128, NCONST], F32, isOutput=False)
    out_d = nc.declare_dram_parameter("outT", [H, S], F32, isOutput=True)

    # feature-major DRAM views: (p, i, n) = W[i*128+p, n]
    def fmaj(d):
        return d.rearrange("(i p) n -> p i n", p=128)

    def layer_norm(tc, nc, pspool, tmp, c_sb, ones_sum, eps_sb, src, apply_out):
        """Normalize src [128, CH, S] over features; call apply_out(jj, nrm)."""
        sum_ps = pspool.tile([128, S], F32, tag="lnsum", bufs=1)
        sq_ps = pspool.tile([128, S], F32, tag="lnsq", bufs=1)
        for i in range(CH):
            nc.tensor.matmul(
                sum_ps[:, :],
                ones_sum[:, :],
                src[:, i, :],
                start=(i == 0),
                stop=(i == CH - 1),
            )
        for i in range(CH):
            sq = tmp.tile([128, S], F32R, tag="sq", bufs=2, name="sq")
            nc.scalar.activation(out=sq, in_=src[:, i, :], func=AF.Square)
            nc.tensor.matmul(
                sq_ps[:, :],
                ones_sum[:, :],
                sq,
                start=(i == 0),
                stop=(i == CH - 1),
            )
        mean = tmp.tile([128, S], F32, tag="mean", bufs=1, name="mean")
        nc.vector.tensor_scalar_mul(mean, sum_ps[:, :], 1.0 / H)
        negm2 = tmp.tile([128, S], F32, tag="negm2", bufs=1, name="negm2")
        nc.vector.scalar_tensor_tensor(
            out=negm2, in0=mean, scalar=-1.0, in1=mean, op0=ALU.mult, op1=ALU.mult
        )
        var = tmp.tile([128, S], F32, tag="var", bufs=1, name="var")
        nc.vector.scalar_tensor_tensor(
            out=var, in0=sq_ps[:, :], scalar=1.0 / H, in1=negm2,
            op0=ALU.mult, op1=ALU.add,
        )
        sd = tmp.tile([128, S], F32, tag="sd", bufs=1, name="sd")
        nc.scalar.activation(out=sd, in_=var, func=AF.Sqrt, bias=eps_sb[:, :])
        rstd = tmp.tile([128, S], F32, tag="rstd", bufs=1, name="rstd")
        nc.vector.reciprocal_approx_fast(out=rstd, in_=sd)
        for jj in range(CH):
            cen = tmp.tile([128, S], F32, tag="cen", bufs=2, name="cen")
            nc.vector.tensor_tensor(
                out=cen, in0=src[:, jj, :], in1=mean, op=ALU.subtract
            )
            nrm = tmp.tile([128, S], F32, tag="nrm", bufs=2, name="nrm")
            nc.vector.tensor_tensor(out=nrm, in0=cen, in1=rstd, op=ALU.mult)
            apply_out(jj, nrm)

    with tile.TileContext(nc) as tc, ExitStack() as top:
        cpool = top.enter_context(tc.tile_pool(name="cpool", bufs=1))
        c_sb = cpool.tile([128, NCONST], F32, name="c_sb")
        nc.sync.dma_start(out=c_sb, in_=c_d[:, :])
        ones_blk = cpool.tile([128, 64], BF16, name="ones_blk")
        nc.vector.memset(ones_blk, 1.0)
        ones_f32 = cpool.tile([128, 128], F32, name="ones_f32")
        nc.vector.memset(ones_f32, 1.0)
        ones_sum = cpool.tile([128, 128], F32R, name="ones_sum")
        nc.vector.tensor_copy(out=ones_sum, in_=ones_f32)
        eps_sb = cpool.tile([128, 1], F32, name="eps_sb")
        nc.vector.memset(eps_sb, EPS)

        mid = top.enter_context(tc.tile_pool(name="mid", bufs=1))
        tmp = top.enter_context(tc.tile_pool(name="tmp", bufs=1))

        for _rep in range(repeats):
            with ExitStack() as s_ac:
                apool = s_ac.enter_context(tc.tile_pool(name="apool", bufs=1))
                xT = apool.tile([128, CH, S], F32R, name="xT")
                for _i in range(CH):
                    nc.sync.dma_start(
                        out=xT[:, _i, :], in_=fmaj(xT_d)[:, _i, :]
                    )
                qT = apool.tile([128, CH, S], F32R, name="qT")
                kT = apool.tile([128, CH, S], F32R, name="kT")
                v_bf = apool.tile([128, T, NH, DH], BF16, name="v_bf")
                ctxT = apool.tile([128, CH, S], F32R, name="ctxT")

                wopool = s_ac.enter_context(tc.tile_pool(name="wopool", bufs=1))
                wo_sb = wopool.tile([128, CH, H], F32R, name="wo_sb")

                # ---- Fused phase A+B: V first, then per head pair:
                # project Q_j/K_j -> S^T -> exp -> denom -> ctx.  exp (ACT)
                # overlaps the next pair's projections (PE).
                with ExitStack() as s_a:
                    wqkv = s_a.enter_context(tc.tile_pool(name="wqkv", bufs=1))
                    wv_sb = wqkv.tile([128, CH, H], F32R, name="wv_sb")
                    nc.sync.dma_start(out=wv_sb, in_=fmaj(wv_d))
                    wqkpool = s_a.enter_context(
                        tc.tile_pool(name="wqkpool", bufs=3)
                    )
                    psA = s_a.enter_context(
                        tc.tile_pool(name="psA", bufs=1, space="PSUM")
                    )
                    bpool = s_a.enter_context(tc.tile_pool(name="bpool", bufs=1))

                    def project_qk(j):
                        for blk_d, dest, bcol in ((wqb_d, qT, BQ), (wkb_d, kT, BK)):
                            wt = wqkpool.tile([128, CH, 128], F32R, tag="wqk",
                                              name="wt")
                            nc.sync.dma_start(out=wt, in_=blk_d[j])
                            ps = psA.tile([128, S], F32, tag="mm", bufs=2,
                                          name="psqk")
                            for i in range(CH):
                                nc.tensor.matmul(
                                    ps[:, :],
                                    wt[:, i, :],
                                    xT[:, i, :],
                                    start=(i == 0),
                                    stop=(i == CH - 1),
                                )
                            nc.scalar.activation(
                                out=dest[:, j, :],
                                in_=ps[:, :],
                                func=AF.Identity,
                                bias=c_sb[:, bcol + j : bcol + j + 1],
                            )

                    project_qk(0)
                    for t in range(T):
                        for half in range(2):
                            ps = psA.tile([128, 384], F32, tag="mmv", bufs=1, name="psv")
                            for i in range(CH):
                                nc.tensor.matmul(
                                    ps[:, :],
                                    xT[:, i, ts(t, 128)],
                                    wv_sb[:, i, ts(half, 384)],
                                    start=(i == 0),
                                    stop=(i == CH - 1),
                                )
                            # v_bf[tok, t, head, d]: cols half*384.. = heads 6h..6h+5
                            nc.vector.tensor_copy(
                                out=v_bf[:, t, ts(half, 6), :].rearrange(
                                    "p h d -> p (h d)"
                                ),
                                in_=ps[:, :],
                            )

                    for j in range(NP):
                        if j > 0:
                            project_qk(j)
                        if j == 2:
                            nc.sync.dma_start(out=wo_sb, in_=fmaj(wo_d))
                        expS_a = bpool.tile([128, T, S], BF16, tag="esa", bufs=2,
                                            name="expS_a")
                        expS_b = bpool.tile([128, T, S], BF16, tag="esb", bufs=2,
                                            name="expS_b")
                        for t in range(T):
                            for half, es in ((0, expS_a), (1, expS_b)):
                                ps = psA.tile([128, S], F32, tag="sT", bufs=2,
                                              name="ps_sT")
                                nc.tensor.matmul(
                                    ps[:, :],
                                    kT[ts(half, 64), j, ts(t, 128)],
                                    qT[ts(half, 64), j, :],
                                    start=True,
                                    stop=True,
                                    tile_position=(half * 64, 0),
                                )
                                nc.scalar.activation(
                                    out=es[:, t, :],
                                    in_=ps[:, :],
                                    func=AF.Exp,
                                    bias=c_sb[:, MB + t : MB + t + 1],
                                    scale=0.125,
                                )
                        # denominators as broadcast rows: [0:64]=a, [64:128]=b
                        den_ps = psA.tile([128, S], F32, tag="den", bufs=1,
                                          name="den_ps")
                        for half, es in ((0, expS_a), (1, expS_b)):
                            for t in range(T):
                                nc.tensor.matmul(
                                    den_ps[ts(half, 64), :],
                                    ones_blk[:, :],
                                    es[:, t, :],
                                    start=(t == 0),
                                    stop=(t == T - 1),
                                    tile_position=(0, half * 64),
                                )
                        recip = bpool.tile([128, S], F32, tag="recip", bufs=2,
                                           name="recip")
                        nc.vector.reciprocal_approx_fast(out=recip, in_=den_ps[:, :])
                        # ctx^T for the pair: head a -> rows 0:64, b -> rows 64:128
                        ctx_ps = psA.tile([128, S], F32, tag="ctx", bufs=2,
                                          name="ctx_ps")
                        for half, es in ((0, expS_a), (1, expS_b)):
                            for t in range(T):
                                nc.tensor.matmul(
                                    ctx_ps[ts(half, 64), :],
                                    v_bf[:, t, 2 * j + half, :],
                                    es[:, t, :],
                                    start=(t == 0),
                                    stop=(t == T - 1),
                                    tile_position=(0, half * 64),
                                )
                        nc.vector.tensor_tensor(
                            out=ctxT[:, j, :], in0=ctx_ps[:, :], in1=recip,
                            op=ALU.mult,
                        )

                # ---- Phase C: Wo^T @ ctx^T, +bo_eff, +x residual, LN1 ----
                r1T = mid.tile([128, CH, S], F32R, name="r1T")
                h1T = mid.tile([128, CH, S], F32, name="h1T")
                h1T_bf = mid.tile([128, CH, S], BF16, name="h1T_bf")
                with ExitStack() as s_c:
                    psC = s_c.enter_context(
                        tc.tile_pool(name="psC", bufs=1, space="PSUM")
                    )
                    for j in range(CH):
                        ps = psC.tile([128, S], F32, tag="mm", bufs=3, name="ps_wo")
                        for i in range(CH):
                            nc.tensor.matmul(
                                ps[:, :],
                                wo_sb[:, i, ts(j, 128)],
                                ctxT[:, i, :],
                                start=(i == 0),
                                stop=(i == CH - 1),
                            )
                        nc.vector.scalar_tensor_tensor(
                            out=r1T[:, j, :],
                            in0=ps[:, :],
                            scalar=c_sb[:, BO + j : BO + j + 1],
                            in1=xT[:, j, :],
                            op0=ALU.add,
                            op1=ALU.add,
                        )

                    def apply_ln1(jj, nrm):
                        nc.scalar.activation(
                            out=h1T[:, jj, :],
                            in_=nrm,
                            func=AF.Identity,
                            bias=c_sb[:, L1B + jj : L1B + jj + 1],
                            scale=c_sb[:, L1G + jj : L1G + jj + 1],
                        )
                        nc.vector.tensor_copy(
                            out=h1T_bf[:, jj, :], in_=h1T[:, jj, :]
                        )

                    layer_norm(tc, nc, psC, tmp, c_sb, ones_sum, eps_sb, r1T, apply_ln1)

            # ---- Phase D: ff1 = gelu(W1^T @ h1^T + b1), bf16 ----
            with ExitStack() as s_de:
                fpool = s_de.enter_context(tc.tile_pool(name="fpool", bufs=1))
                geluT = fpool.tile([128, CF, S], BF16, name="geluT")
                w1pool = s_de.enter_context(tc.tile_pool(name="w1pool", bufs=6))
                psD = s_de.enter_context(tc.tile_pool(name="psD", bufs=1, space="PSUM"))
                for f in range(CF):
                    w1t = w1pool.tile([128, CH, 128], BF16, tag="w1", name="w1t")
                    nc.sync.dma_start(out=w1t, in_=w1_d[f])
                    ps = psD.tile([128, S], F32, tag="mm", bufs=4, name="ps_f1")
                    for i in range(CH):
                        nc.tensor.matmul(
                            ps[:, :],
                            w1t[:, i, :],
                            h1T_bf[:, i, :],
                            start=(i == 0),
                            stop=(i == CH - 1),
                        )
                    if gelu_mode == "hw":
                        nc.scalar.activation(
                            out=geluT[:, f, :],
                            in_=ps[:, :],
                            func=AF.Gelu,
                            bias=c_sb[:, B1 + f : B1 + f + 1],
                        )
                    else:
                        # tanh approx: 0.5*x*(1+tanh(0.79788456*(x+0.044715*x^3)))
                        xf = fpool.tile([128, S], F32, tag="gx", bufs=2, name="gx")
                        nc.scalar.activation(
                            out=xf, in_=ps[:, :], func=AF.Identity,
                            bias=c_sb[:, B1 + f : B1 + f + 1],
                        )
                        sqg = fpool.tile([128, S], F32, tag="gsq", bufs=2, name="gsq")
                        nc.vector.tensor_tensor(out=sqg, in0=xf, in1=xf, op=ALU.mult)
                        vg = fpool.tile([128, S], F32, tag="gv", bufs=2, name="gv")
                        nc.vector.tensor_scalar(
                            out=vg, in0=sqg, scalar1=0.044715, scalar2=1.0,
                            op0=ALU.mult, op1=ALU.add,
                        )
                        ug = fpool.tile([128, S], F32, tag="gu", bufs=2, name="gu")
                        nc.vector.tensor_tensor(out=ug, in0=xf, in1=vg, op=ALU.mult)
                        tg = fpool.tile([128, S], F32, tag="gt", bufs=2, name="gt")
                        nc.scalar.activation(
                            out=tg, in_=ug, func=AF.Tanh, scale=0.7978845608
                        )
                        wg = fpool.tile([128, S], F32, tag="gw", bufs=2, name="gw")
                        nc.vector.scalar_tensor_tensor(
                            out=wg, in0=tg, scalar=1.0, in1=xf,
                            op0=ALU.add, op1=ALU.mult,
                        )
                        nc.vector.tensor_scalar_mul(geluT[:, f, :], wg, 0.5)

                # ---- Phase E: ff2 = W2^T @ gelu^T + b2, +h1 residual, LN2 ----
                r2T = mid.tile([128, CH, S], F32R, name="r2T")
                w2pool = s_de.enter_context(tc.tile_pool(name="w2pool", bufs=3))
                for j in range(CH):
                    w2t = w2pool.tile([128, CF, 128], BF16, tag="w2", name="w2t")
                    nc.sync.dma_start(out=w2t, in_=w2_d[j])
                    ps = psD.tile([128, S], F32, tag="mm", bufs=4, name="ps_f2")
                    for i in range(CF):
                        nc.tensor.matmul(
                            ps[:, :],
                            w2t[:, i, :],
                            geluT[:, i, :],
                            start=(i == 0),
                            stop=(i == CF - 1),
                        )
                    nc.vector.scalar_tensor_tensor(
                        out=r2T[:, j, :],
                        in0=ps[:, :],
                        scalar=c_sb[:, B2 + j : B2 + j + 1],
                        in1=h1T[:, j, :],
                        op0=ALU.add,
                        op1=ALU.add,
                    )

                def apply_ln2(jj, nrm):
                    ot = tmp.tile([128, S], F32, tag="ot", bufs=2, name="ot")
                    nc.scalar.activation(
                        out=ot,
                        in_=nrm,
                        func=AF.Identity,
                        bias=c_sb[:, L2B + jj : L2B + jj + 1],
                        scale=c_sb[:, L2G + jj : L2G + jj + 1],
                    )
                    nc.sync.dma_start(out=out_d[ts(jj, 128), :], in_=ot)

                layer_norm(tc, nc, psD, tmp, c_sb, ones_sum, eps_sb, r2T, apply_ln2)

    nc.finalize()
    return nc


_NC_CACHE = None


def _get_nc():
    global _NC_CACHE
    if _NC_CACHE is None:
        _NC_CACHE = build_nc()
    return _NC_CACHE


def make_in_maps(hidden_states, attention_mask, Wq, bq, Wk, bk, Wv, bv, Wo, bo,
                 ln1_g, ln1_b, W1, b1, W2, b2, ln2_g, ln2_b):
    """Host-side sharding + layout prep. Returns one input map per core."""
    f32 = np.float32
    bf16 = ml_dtypes.bfloat16
    Wq, Wk, Wv, Wo = (np.asarray(w, f32) for w in (Wq, Wk, Wv, Wo))
    W1, W2 = np.asarray(W1, f32), np.asarray(W2, f32)
    bo_eff = np.asarray(bo, f32) + np.asarray(bv, f32) @ Wo

    wqb = np.ascontiguousarray(Wq.reshape(CH, 128, CH, 128).transpose(2, 1, 0, 3))
    wkb = np.ascontiguousarray(Wk.reshape(CH, 128, CH, 128).transpose(2, 1, 0, 3))
    w1b = np.ascontiguousarray(
        W1.reshape(CH, 128, CF, 128).transpose(2, 1, 0, 3)
    ).astype(bf16)
    w2b = np.ascontiguousarray(
        W2.reshape(CF, 128, CH, 128).transpose(2, 1, 0, 3)
    ).astype(bf16)

    def cols(v, n):
        return np.ascontiguousarray(np.asarray(v, f32).reshape(n, 128).T)

    maskb = (1.0 - np.asarray(attention_mask, f32)) * -10000.0  # [B, S]

    base = np.zeros((128, NCONST), f32)
    base[:, BQ:BQ + CH] = cols(bq, CH)
    base[:, BK:BK + CH] = cols(bk, CH)
    base[:, BO:BO + CH] = cols(bo_eff, CH)
    base[:, L1G:L1G + CH] = cols(ln1_g, CH)
    base[:, L1B:L1B + CH] = cols(ln1_b, CH)
    base[:, B2:B2 + CH] = cols(b2, CH)
    base[:, L2G:L2G + CH] = cols(ln2_g, CH)
    base[:, L2B:L2B + CH] = cols(ln2_b, CH)
    base[:, B1:B1 + CF] = cols(b1, CF)

    x = np.asarray(hidden_states, f32)
    in_maps = []
    for b in range(B):
        consts = base.copy()
        consts[:, MB:MB + T] = cols(maskb[b], T)
        in_maps.append({
            "xT": np.ascontiguousarray(x[b].T),
            "wqb": wqb, "wkb": wkb, "wv": Wv, "wo": Wo,
            "w1b": w1b, "w2b": w2b,
            "consts": consts,
        })
    return in_maps


def kernel(**inputs):
    nc = _get_nc()
    in_maps = make_in_maps(**inputs)
    res = run_bass_kernel_spmd(nc, in_maps, core_ids=list(range(B)))
    out = np.stack([np.ascontiguousarray(r["outT"].T) for r in res.results])
    return out.astype(np.float32)

